# revision 7
# baseline (speedup 1.0000x reference)
"""Batch-softmax dot-product attention on 8 trn2 NeuronCores — v2.

reference:  S = einsum('bqd,bkd->bqk', Q, K) / sqrt(D)
            A = softmax(S, axis=0)            # over the BATCH dim!
            out = einsum('bqk,bkd->bqd', A, V)

Sharding: split the QUERY dim across the 8 cores (256 queries each);
all 16 batches resident per core => no collectives.

v2 vs v1 (HW-A/B-validated, ~10% faster): the v1 kernel was not
engine-throughput-bound — per-engine busy was PE 84 / DVE 71 / ACT 61
out of 112us total (TimelineSim), so the rebalance targets the max
engine and the stall structure:
  - zsum moved OFF the PE (was 128 ident-matmuls, 27us) onto a DVE
    bf16 pairwise add tree (tensor_tensor add runs 2x on packed bf16).
  - the freed psZ PSUM bank doubles the B exp slot: 4 uniform 4-batch
    slots per kt-tile (64 1KB ACT exp instrs instead of 80 mixed).
  - all exp on ACT (v1's custom-DVE exp32 slots removed: DVE was the
    fuller engine).
  - reciprocal_approx_fast emitted straight to bf16 (drops the NR
    refinement pass and the f32->bf16 copy; ~18 bits, gate is 2e-2).
  - V/ident DMAs on the ACT HWDGE queue, K/Q/out on the SP queue.
  - group-0 K/V pinned in SBUF and re-DMA'd at body end so the next
    repeat iteration starts with data resident (6/7 paired reps
    faster, ~-1.2us; dead weight overlapping the out DMA for the
    single-shot run).
  - output staged and DMA'd as bf16 (halves the out traffic and the
    final copy; host upcasts to f32; adds ~2e-3 rel err, gate is 2e-2;
    4/6 paired HW reps faster, ~-5us).
  - norm/mul/mm2 pipelined at t-half granularity ("tsplit", 8/8 paired
    HW reps faster, median -8us): per g the emission is
      S1(g) | S3(g-2,t0) S4(g-2,t0) | S2(g-1,t0) | S3(g-2,t1)
      S4(g-2,t1) | S2(g-1,t1)
    which keeps ready work at the head of the PE and DVE queues and
    shortens every cross-engine dependency chain.

Per-group stages (k-major layout, k on partitions):
  S1(g): DMA K/V, mm1 S^T = K^T.Q into PSUM slots, ACT exp -> P bf16
  S2(g): zsum = DVE add tree over b -> Z f32, recip -> R bf16
  S3(g): A = P * R (DVE, R broadcast over b)
  S4(g): mm2 outT += V^T.A, PSUM-accumulated over all 16 kt tiles

Engine busy (sim, per core): ACT 68 (exp 54.6 elem + instr ovh), DVE
~62 (mul 34 + tree 17 + recip/copies), PE 58 (mm1 27 + mm2 27 + LS),
GPSIMD idle (measured 2x slower than its cost model on tensor ops -
slot_gp/l1_gp variants exist in CFG but lose on HW).

CFG holds the A/B knobs; defaults are the HW-fastest combination.
"""

import numpy as np
import ml_dtypes

import concourse.bass as bass
import concourse.bacc as bacc
import concourse.tile as tile_mod
from concourse import mybir
from concourse.bass_utils import run_bass_kernel_spmd

B, N, D = 16, 2048, 64
NCORES = 8
QL = N // NCORES           # 256 queries per core
KT = 128                   # keys per kt tile
TK = 2                     # kt tiles per group
NG = N // (KT * TK)        # 8 groups
BP = B // 2                # 8 batch pairs
GK = TK * KT               # 256 keys per group
ZPE = 8                    # batches zsummed on PE; the rest on DVE tree
BF = mybir.dt.bfloat16
F32 = mybir.dt.float32
SCALE = 1.0 / np.sqrt(D)

bf16 = ml_dtypes.bfloat16

# mm1/exp slot schedule per kt tile (from v1): each PSUM bank gets a
# uniform-parity batch pair (T0/T8 row-tiling constraint); B slot first
# covers ACT's bubble at t/g boundaries.
SLOT_SCHED = [
    ("B", [4, 6]),
    ("A", [0, 2, 1, 3]),
    ("A", [5, 7, 8, 10]),
    ("B", [9, 11]),
    ("A", [12, 14, 13, 15]),
]
# zpe==0 variant: psZ freed -> psB grows to 2 banks, 4 uniform slots
SLOT_SCHED4 = [
    ("A", [0, 2, 1, 3]),
    ("B", [4, 6, 5, 7]),
    ("A", [8, 10, 9, 11]),
    ("B", [12, 14, 13, 15]),
]


CFG = {"zpe": 0, "recip_bf": True, "order": "s2_first", "tsplit": True, "l1_gp": False, "slot_gp": False, "gp_slots": (), "ktalt": False, "obf": True, "pbuf": 3, "kvbuf": 3, "ilv": False, "g0pin": True}


def build_program(repeat=1):
    nc = bacc.Bacc(trn_type="TRN2")

    qH = nc.dram_tensor("qH", [128, BP, QL], BF, kind="ExternalInput")
    kH = nc.dram_tensor("kH", [NG, 128, BP, GK], BF, kind="ExternalInput")
    vH = nc.dram_tensor("vH", [NG, 128, TK, B, D], BF, kind="ExternalInput")
    outH = nc.dram_tensor("outH", [128, BP, QL],
                          BF if CFG["obf"] else F32,
                          kind="ExternalOutput")

    ident = nc.inline_tensor(np.eye(128, dtype=bf16), name="ident")

    with tile_mod.TileContext(nc) as tc:
        with (
            tc.tile_pool(name="singles", bufs=1) as singles,
            tc.tile_pool(name="kt", bufs=CFG["kvbuf"]) as kt_pool,
            tc.tile_pool(name="v", bufs=CFG["kvbuf"]) as v_pool,
            tc.tile_pool(name="p", bufs=CFG["pbuf"]) as p_pool,
            tc.tile_pool(name="attn", bufs=2) as a_pool,
            tc.tile_pool(name="r", bufs=2) as r_pool,
            tc.tile_pool(name="tree", bufs=1) as tree_pool,
            # PSUM allocation order fixes bank placement.
            tc.tile_pool(name="psA", bufs=1, space="PSUM") as psA_pool,
            tc.tile_pool(name="psB", bufs=1, space="PSUM") as psB_pool,
            tc.tile_pool(name="psZ", bufs=1, space="PSUM") as psZ_pool,
            tc.tile_pool(name="psO", bufs=1, space="PSUM") as psO_pool,
        ):
            qt_sb = singles.tile([128, BP, QL], BF, name="qt_sb")
            nc.sync.dma_start(out=qt_sb, in_=qH[:, :, :])
            id_sb = singles.tile([128, 128], BF, name="id_sb")
            nc.scalar.dma_start(out=id_sb, in_=ident[:, :])

            outacc = [psO_pool.tile([128, 2 * QL], F32, tag=f"o{i}", name=f"outacc{i}")
                      for i in range(BP // 2)]

            if CFG["g0pin"]:
                kt_pin = singles.tile([128, BP, GK], BF, name="kt_pin")
                v_pin = singles.tile([128, TK, B, D], BF, name="v_pin")
                nc.sync.dma_start(out=kt_pin, in_=kH[0])
                nc.scalar.dma_start(out=v_pin, in_=vH[0])

            Ps, As, Vs, Rz, T1s = {}, {}, {}, {}, {}

            def emit_s1(g, chunk=None):
                if CFG["g0pin"] and g == 0:
                    kt_sb, v_sb = kt_pin, v_pin
                else:
                    kt_sb = kt_pool.tile([128, BP, GK], BF, tag="kt",
                                         name=f"kt{g}")
                    v_sb = v_pool.tile([128, TK, B, D], BF, tag="v",
                                       name=f"v{g}")
                    if CFG["ktalt"] and g % 2 == 1:
                        nc.scalar.dma_start(out=kt_sb, in_=kH[g])
                        nc.sync.dma_start(out=v_sb, in_=vH[g])
                    else:
                        nc.sync.dma_start(out=kt_sb, in_=kH[g])
                        nc.scalar.dma_start(out=v_sb, in_=vH[g])
                Vs[g] = v_sb
                P = p_pool.tile([128, B, TK * QL], BF, tag="p", name=f"P{g}")
                Ps[g] = P
                if CFG["slot_gp"]:
                    T1s[g] = tree_pool.tile([128, 8, TK * QL], BF, tag="t1s",
                                            name=f"t1s_{g}")
                BS = TK * QL
                sched = SLOT_SCHED4 if CFG["zpe"] == 0 else SLOT_SCHED
                for t in range(TK):
                    for slot, bl in sched:
                        nb = len(bl)
                        if slot == "A":
                            s_ps = psA_pool.tile([128, 4 * QL], F32, tag="sa",
                                                 name=f"sa{g}_{t}")
                        elif CFG["zpe"] == 0:
                            s_ps = psB_pool.tile([128, 4 * QL], F32, tag="sb",
                                                 name=f"sb{g}_{t}")
                        else:
                            s_ps = psB_pool.tile([128, 2 * QL], F32, tag="sb",
                                                 name=f"sb{g}_{t}")
                        for i, b in enumerate(bl):
                            bo, bp = b % 2, b // 2
                            nc.tensor.matmul(
                                out=s_ps[:, i * QL:(i + 1) * QL],
                                lhsT=kt_sb[bo * 64:(bo + 1) * 64, bp,
                                           t * KT:(t + 1) * KT],
                                rhs=qt_sb[bo * 64:(bo + 1) * 64, bp, :],
                                start=(i % 2 == 0), stop=(i % 2 == 1),
                            )
                        p_ap = P[:, :, :]
                        off = p_ap.offset + bl[0] * BS + t * QL
                        if nb == 4:
                            free = [[(bl[2] - bl[0]) * BS, 2],
                                    [(bl[1] - bl[0]) * BS, 2], [1, QL]]
                            in_ap = s_ps[:, :].rearrange(
                                "p (o i q) -> p o i q", o=2, i=2)
                        else:
                            free = [[(bl[1] - bl[0]) * BS, 2], [1, QL]]
                            in_ap = s_ps[:, :].rearrange(
                                "p (i q) -> p i q", i=2)
                        out_ap = bass.AP(tensor=p_ap.tensor, offset=off,
                                         ap=[p_ap.ap[0]] + free)
                        nc.scalar.activation(
                            out=out_ap, in_=in_ap,
                            func=mybir.ActivationFunctionType.Exp,
                            scale=SCALE,
                        )
                        if CFG["slot_gp"]:
                            # pair-add this slot's 4 batches; GPSIMD for
                            # slots in gp_slots, DVE otherwise
                            si = sched.index((slot, bl))
                            tcs = slice(t * QL, (t + 1) * QL)
                            a = bl[0]
                            eng = (nc.gpsimd if si in CFG["gp_slots"]
                                   else nc.vector)
                            eng.tensor_tensor(
                                out=T1s[g][:, 2 * si:2 * si + 2, tcs],
                                in0=P[:, a:a + 2, tcs],
                                in1=P[:, a + 2:a + 4, tcs],
                                op=mybir.AluOpType.add)
                        if chunk is not None:
                            chunk()

            def emit_s2(g, lo=0, w=TK * QL, tag=""):
                """zsum + recip for columns [lo, lo+w) of group g -> Rb."""
                P = Ps[g]
                cs = slice(lo, lo + w)
                ZPEc = CFG["zpe"]
                m = B - ZPEc
                if CFG["slot_gp"]:
                    t1 = T1s[g][:, :, cs]
                else:
                    t1t = tree_pool.tile([128, m // 2, w], BF, tag=f"t1{tag}",
                                         name=f"t1{tag}_{g}")
                    l1_eng = nc.gpsimd if CFG["l1_gp"] else nc.vector
                    l1_eng.tensor_tensor(
                        out=t1t[:, :, :], in0=P[:, ZPEc::2, cs],
                        in1=P[:, ZPEc + 1::2, cs], op=mybir.AluOpType.add)
                    t1 = t1t[:, :, :]
                lvl, lw, li = t1, m // 2, 2
                stop_w = 2 if ZPEc == 0 else 1
                while lw > stop_w:
                    nxt = tree_pool.tile([128, lw // 2, w], BF,
                                         tag=f"t{li}{tag}",
                                         name=f"t{li}{tag}_{g}")
                    nc.vector.tensor_tensor(
                        out=nxt[:, :, :], in0=lvl[:, 0::2, :],
                        in1=lvl[:, 1::2, :], op=mybir.AluOpType.add)
                    lvl, lw, li = nxt[:, :, :], lw // 2, li + 1
                Zf = r_pool.tile([128, w], F32, tag=f"zf{tag}",
                                 name=f"zf{tag}_{g}")
                if ZPEc == 0:
                    nc.vector.tensor_tensor(out=Zf[:, :], in0=lvl[:, 0, :],
                                            in1=lvl[:, 1, :],
                                            op=mybir.AluOpType.add)
                else:
                    Zp = psZ_pool.tile([128, TK * QL], F32, tag="z",
                                       name=f"z{tag}{g}")
                    for j in range(ZPEc):
                        nc.tensor.matmul(
                            out=Zp[:, cs], lhsT=id_sb[:, :], rhs=P[:, j, cs],
                            start=(j == 0), stop=(j == ZPEc - 1),
                        )
                    nc.vector.tensor_tensor(out=Zf[:, :], in0=Zp[:, cs],
                                            in1=lvl[:, 0, :],
                                            op=mybir.AluOpType.add)
                Rb = r_pool.tile([128, w], BF, tag=f"rb{tag}",
                                 name=f"rb{tag}_{g}")
                if CFG["recip_bf"]:
                    from concourse.dve_ops import (
                        RECIP_APPROX_FAST_CONSTS, RECIPROCAL_APPROX_FAST)
                    c = RECIP_APPROX_FAST_CONSTS
                    nc.vector._custom_dve(
                        RECIPROCAL_APPROX_FAST, out=Rb[:, :], in0=Zf[:, :],
                        s0=c["s0"], s1=c["s1"], imm2=c["imm2"])
                else:
                    Rf = r_pool.tile([128, w], F32, tag=f"rf{tag}",
                                     name=f"rf{tag}_{g}")
                    nc.vector.reciprocal_approx_fast(out=Rf[:, :], in_=Zf[:, :])
                    nc.vector.tensor_copy(out=Rb[:, :], in_=Rf[:, :])
                return Rb

            def emit_s3(g, Rb, lo=0, w=TK * QL):
                P = Ps[g]
                cs = slice(lo, lo + w)
                A = As.get(g)
                if A is None:
                    A = a_pool.tile([128, B, TK * QL], BF, tag="a",
                                    name=f"A{g}")
                    As[g] = A
                rb_ap = Rb[:, :]
                rb_bcast = bass.AP(tensor=rb_ap.tensor, offset=rb_ap.offset,
                                   ap=[rb_ap.ap[0], [0, B], rb_ap.ap[1]])
                nc.vector.tensor_mul(out=A[:, :, cs], in0=P[:, :, cs],
                                     in1=rb_bcast)

            def s4_matmuls(g, first, last, ts=tuple(range(TK))):
                A, v_sb = As[g], Vs[g]
                for b in range(B):
                    bo, bp = b % 2, b // 2
                    for t in ts:
                        yield dict(
                            out=outacc[bp // 2][bo * 64:(bo + 1) * 64,
                                                (bp % 2) * QL:(bp % 2 + 1) * QL],
                            lhsT=v_sb[:, t, b, :],
                            rhs=A[:, b, t * QL:(t + 1) * QL],
                            start=(first and t == 0 and bp % 2 == 0),
                            stop=(last and t == TK - 1 and bp % 2 == 1),
                            # CoreSim's group tracker is partition-base
                            # blind; data semantics verified separately.
                            skip_group_check=True,
                        )

            def emit_s4(g, first, last, ts=tuple(range(TK))):
                for kw in s4_matmuls(g, first, last, ts):
                    nc.tensor.matmul(**kw)

            def release(g):
                Ps.pop(g, None), As.pop(g, None), Vs.pop(g, None)
                T1s.pop(g, None)

            import contextlib
            rep_ctx = tc.For_i(0, repeat, 1) if repeat > 1 else contextlib.nullcontext()
            with rep_ctx:
                if CFG["ilv"]:
                    for g in range(NG):
                        if g >= 2:
                            emit_s3(g - 2, Rz.pop(g - 2))
                            mm2_iter = s4_matmuls(g - 2, first=(g == 2),
                                                  last=False)
                            def chunk(it=mm2_iter):
                                for _ in range(4):
                                    kw = next(it, None)
                                    if kw is not None:
                                        nc.tensor.matmul(**kw)
                            emit_s1(g, chunk=chunk)
                            for kw in mm2_iter:
                                nc.tensor.matmul(**kw)
                            release(g - 2)
                        else:
                            emit_s1(g)
                        if g >= 1:
                            Rz[g - 1] = emit_s2(g - 1)
                    GL = NG - 1
                    emit_s3(GL - 1, Rz.pop(GL - 1))
                    emit_s4(GL - 1, first=False, last=False)
                    release(GL - 1)
                    Rb0 = emit_s2(GL, lo=0, w=QL, tag="h0")
                    emit_s3(GL, Rb0, lo=0, w=QL)
                    emit_s4(GL, first=False, last=False, ts=(0,))
                    Rb1 = emit_s2(GL, lo=QL, w=QL, tag="h1")
                    emit_s3(GL, Rb1, lo=QL, w=QL)
                    emit_s4(GL, first=False, last=True, ts=(1,))
                    release(GL)
                elif not CFG["tsplit"]:
                    for g in range(NG):
                        emit_s1(g)
                        if CFG["order"] == "s34_first":
                            if g >= 2:
                                emit_s3(g - 2, Rz.pop(g - 2))
                                emit_s4(g - 2, first=(g == 2), last=False)
                                release(g - 2)
                            if g >= 1:
                                Rz[g - 1] = emit_s2(g - 1)
                        else:
                            if g >= 1:
                                Rz[g - 1] = emit_s2(g - 1)
                            if g >= 2:
                                emit_s3(g - 2, Rz.pop(g - 2))
                                emit_s4(g - 2, first=(g == 2), last=False)
                                release(g - 2)
                    GL = NG - 1
                    emit_s3(GL - 1, Rz.pop(GL - 1))
                    emit_s4(GL - 1, first=False, last=False)
                    release(GL - 1)
                    Rb0 = emit_s2(GL, lo=0, w=QL, tag="h0")
                    emit_s3(GL, Rb0, lo=0, w=QL)
                    emit_s4(GL, first=False, last=False, ts=(0,))
                    Rb1 = emit_s2(GL, lo=QL, w=QL, tag="h1")
                    emit_s3(GL, Rb1, lo=QL, w=QL)
                    emit_s4(GL, first=False, last=True, ts=(1,))
                    release(GL)
                else:
                    # unit pipeline over (g, th): S2/S3/S4 at t-half grain
                    def s2u(g, th):
                        Rz[(g, th)] = emit_s2(g, lo=th * QL, w=QL,
                                              tag=f"u{th}")
                    def s34u(g, th, first, last):
                        emit_s3(g, Rz.pop((g, th)), lo=th * QL, w=QL)
                        emit_s4(g, first=first, last=last, ts=(th,))
                    for g in range(NG):
                        emit_s1(g)
                        if g >= 2:
                            s34u(g - 2, 0, first=(g == 2), last=False)
                        if g >= 1:
                            s2u(g - 1, 0)
                        if g >= 2:
                            s34u(g - 2, 1, first=(g == 2), last=False)
                        if g >= 1:
                            s2u(g - 1, 1)
                    GL = NG - 1
                    s34u(GL - 1, 0, first=False, last=False)
                    s2u(GL, 0)
                    s34u(GL - 1, 1, first=False, last=False)
                    s2u(GL, 1)
                    s34u(GL, 0, first=False, last=False)
                    s34u(GL, 1, first=False, last=True)
                    for g in range(NG):
                        release(g)

                out_sb = singles.tile([128, BP, QL],
                                      BF if CFG["obf"] else F32,
                                      name="out_sb")
                for i in range(BP // 2):
                    nc.vector.tensor_copy(
                        out=out_sb[:, 2 * i:2 * i + 2, :],
                        in_=outacc[i][:, :].rearrange("p (j q) -> p j q", j=2),
                    )
                nc.sync.dma_start(out=outH[:, :, :], in_=out_sb)
                if CFG["g0pin"]:
                    # refill g0's pinned K/V for the next repeat iteration;
                    # the DMA flies across the loop barrier. Dead weight for
                    # the single-shot run (overlaps the out DMA).
                    nc.sync.dma_start(out=kt_pin, in_=kH[0])
                    nc.scalar.dma_start(out=v_pin, in_=vH[0])

    nc.finalize()
    return nc


_NC_CACHE = None


def _get_program():
    global _NC_CACHE
    if _NC_CACHE is None:
        _NC_CACHE = build_program()
    return _NC_CACHE


def make_in_maps(queries, keys, values):
    """Host-side staging into SBUF partition-images (bf16)."""
    kHt = np.ascontiguousarray(
        keys.reshape(BP, 2, NG, GK, D).transpose(2, 1, 4, 0, 3)
    ).reshape(NG, 128, BP, GK).astype(bf16)
    vHt = np.ascontiguousarray(
        values.reshape(B, NG, TK, KT, D).transpose(1, 3, 2, 0, 4)
    ).astype(bf16)
    in_maps = []
    for c in range(NCORES):
        qs = queries[:, c * QL:(c + 1) * QL, :]
        qHc = np.ascontiguousarray(
            qs.reshape(BP, 2, QL, D).transpose(1, 3, 0, 2)
        ).reshape(128, BP, QL).astype(bf16)
        in_maps.append({"qH": qHc, "kH": kHt, "vH": vHt})
    return in_maps


def assemble_output(results):
    out = np.empty((B, N, D), dtype=np.float32)
    for c, res in enumerate(results):
        oc = res["outH"].astype(np.float32)
        oc = oc.reshape(2, D, BP, QL).transpose(2, 0, 3, 1)
        out[:, c * QL:(c + 1) * QL, :] = oc.reshape(B, QL, D)
    return out


def kernel(queries, keys, values):
    nc = _get_program()
    in_maps = make_in_maps(queries, keys, values)
    res = run_bass_kernel_spmd(nc, in_maps, core_ids=list(range(NCORES)))
    return assemble_output(res.results)


# revision 8
# speedup vs baseline: 1.0089x; 1.0089x over previous
"""Batch-softmax dot-product attention on 8 trn2 NeuronCores — v2.

reference:  S = einsum('bqd,bkd->bqk', Q, K) / sqrt(D)
            A = softmax(S, axis=0)            # over the BATCH dim!
            out = einsum('bqk,bkd->bqd', A, V)

Sharding: split the QUERY dim across the 8 cores (256 queries each);
all 16 batches resident per core => no collectives.

v2 vs v1 (HW-A/B-validated, ~10% faster): the v1 kernel was not
engine-throughput-bound — per-engine busy was PE 84 / DVE 71 / ACT 61
out of 112us total (TimelineSim), so the rebalance targets the max
engine and the stall structure:
  - zsum moved OFF the PE (was 128 ident-matmuls, 27us) onto a DVE
    bf16 pairwise add tree (tensor_tensor add runs 2x on packed bf16).
  - the freed psZ PSUM bank doubles the B exp slot: 4 uniform 4-batch
    slots per kt-tile (64 1KB ACT exp instrs instead of 80 mixed).
  - all exp on ACT (v1's custom-DVE exp32 slots removed: DVE was the
    fuller engine).
  - reciprocal_approx_fast emitted straight to bf16 (drops the NR
    refinement pass and the f32->bf16 copy; ~18 bits, gate is 2e-2).
  - V/ident DMAs on the ACT HWDGE queue, K/Q/out on the SP queue.
  - group-0 K/V pinned in SBUF and re-DMA'd at body end so the next
    repeat iteration starts with data resident (6/7 paired reps
    faster, ~-1.2us; dead weight overlapping the out DMA for the
    single-shot run).
  - output staged and DMA'd as bf16 (halves the out traffic and the
    final copy; host upcasts to f32; adds ~2e-3 rel err, gate is 2e-2;
    4/6 paired HW reps faster, ~-5us).
  - norm/mul/mm2 pipelined at t-half granularity ("tsplit", 8/8 paired
    HW reps faster, median -8us): per g the emission is
      S1(g) | S3(g-2,t0) S4(g-2,t0) | S2(g-1,t0) | S3(g-2,t1)
      S4(g-2,t1) | S2(g-1,t1)
    which keeps ready work at the head of the PE and DVE queues and
    shortens every cross-engine dependency chain.

Per-group stages (k-major layout, k on partitions):
  S1(g): DMA K/V, mm1 S^T = K^T.Q into PSUM slots, ACT exp -> P bf16
  S2(g): zsum = DVE add tree over b -> Z f32, recip -> R bf16
  S3(g): A = P * R (DVE, R broadcast over b)
  S4(g): mm2 outT += V^T.A, PSUM-accumulated over all 16 kt tiles

Engine busy (sim, per core): ACT 68 (exp 54.6 elem + instr ovh), DVE
~62 (mul 34 + tree 17 + recip/copies), PE 58 (mm1 27 + mm2 27 + LS),
GPSIMD idle (measured 2x slower than its cost model on tensor ops -
slot_gp/l1_gp variants exist in CFG but lose on HW).

CFG holds the A/B knobs; defaults are the HW-fastest combination.
"""

import numpy as np
import ml_dtypes

import concourse.bass as bass
import concourse.bacc as bacc
import concourse.tile as tile_mod
from concourse import mybir
from concourse.bass_utils import run_bass_kernel_spmd

B, N, D = 16, 2048, 64
NCORES = 8
QL = N // NCORES           # 256 queries per core
KT = 128                   # keys per kt tile
TK = 2                     # kt tiles per group
NG = N // (KT * TK)        # 8 groups
BP = B // 2                # 8 batch pairs
GK = TK * KT               # 256 keys per group
ZPE = 8                    # batches zsummed on PE; the rest on DVE tree
BF = mybir.dt.bfloat16
F32 = mybir.dt.float32
SCALE = 1.0 / np.sqrt(D)

bf16 = ml_dtypes.bfloat16

# mm1/exp slot schedule per kt tile (from v1): each PSUM bank gets a
# uniform-parity batch pair (T0/T8 row-tiling constraint); B slot first
# covers ACT's bubble at t/g boundaries.
SLOT_SCHED = [
    ("B", [4, 6]),
    ("A", [0, 2, 1, 3]),
    ("A", [5, 7, 8, 10]),
    ("B", [9, 11]),
    ("A", [12, 14, 13, 15]),
]
# zpe==0 variant: psZ freed -> psB grows to 2 banks, 4 uniform slots
SLOT_SCHED4 = [
    ("A", [0, 2, 1, 3]),
    ("B", [4, 6, 5, 7]),
    ("A", [8, 10, 9, 11]),
    ("B", [12, 14, 13, 15]),
]


CFG = {"zpe": 0, "recip_bf": True, "order": "s2_first", "tsplit": True, "l1_gp": False, "slot_gp": False, "gp_slots": (), "ktalt": False, "obf": True, "pbuf": 4, "kvbuf": 3, "ilv": False, "g0pin": True}


def build_program(repeat=1):
    nc = bacc.Bacc(trn_type="TRN2")

    qH = nc.dram_tensor("qH", [128, BP, QL], BF, kind="ExternalInput")
    kH = nc.dram_tensor("kH", [NG, 128, BP, GK], BF, kind="ExternalInput")
    vH = nc.dram_tensor("vH", [NG, 128, TK, B, D], BF, kind="ExternalInput")
    outH = nc.dram_tensor("outH", [128, BP, QL],
                          BF if CFG["obf"] else F32,
                          kind="ExternalOutput")

    ident = nc.inline_tensor(np.eye(128, dtype=bf16), name="ident")

    with tile_mod.TileContext(nc) as tc:
        with (
            tc.tile_pool(name="singles", bufs=1) as singles,
            tc.tile_pool(name="kt", bufs=CFG["kvbuf"]) as kt_pool,
            tc.tile_pool(name="v", bufs=CFG["kvbuf"]) as v_pool,
            tc.tile_pool(name="p", bufs=CFG["pbuf"]) as p_pool,
            tc.tile_pool(name="attn", bufs=2) as a_pool,
            tc.tile_pool(name="r", bufs=2) as r_pool,
            tc.tile_pool(name="tree", bufs=1) as tree_pool,
            # PSUM allocation order fixes bank placement.
            tc.tile_pool(name="psA", bufs=1, space="PSUM") as psA_pool,
            tc.tile_pool(name="psB", bufs=1, space="PSUM") as psB_pool,
            tc.tile_pool(name="psZ", bufs=1, space="PSUM") as psZ_pool,
            tc.tile_pool(name="psO", bufs=1, space="PSUM") as psO_pool,
        ):
            qt_sb = singles.tile([128, BP, QL], BF, name="qt_sb")
            nc.sync.dma_start(out=qt_sb, in_=qH[:, :, :])
            id_sb = singles.tile([128, 128], BF, name="id_sb")
            nc.scalar.dma_start(out=id_sb, in_=ident[:, :])

            outacc = [psO_pool.tile([128, 2 * QL], F32, tag=f"o{i}", name=f"outacc{i}")
                      for i in range(BP // 2)]

            if CFG["g0pin"]:
                kt_pin = singles.tile([128, BP, GK], BF, name="kt_pin")
                v_pin = singles.tile([128, TK, B, D], BF, name="v_pin")
                nc.sync.dma_start(out=kt_pin, in_=kH[0])
                nc.scalar.dma_start(out=v_pin, in_=vH[0])

            Ps, As, Vs, Rz, T1s = {}, {}, {}, {}, {}

            def emit_s1(g, chunk=None):
                if CFG["g0pin"] and g == 0:
                    kt_sb, v_sb = kt_pin, v_pin
                else:
                    kt_sb = kt_pool.tile([128, BP, GK], BF, tag="kt",
                                         name=f"kt{g}")
                    v_sb = v_pool.tile([128, TK, B, D], BF, tag="v",
                                       name=f"v{g}")
                    if CFG["ktalt"] and g % 2 == 1:
                        nc.scalar.dma_start(out=kt_sb, in_=kH[g])
                        nc.sync.dma_start(out=v_sb, in_=vH[g])
                    else:
                        nc.sync.dma_start(out=kt_sb, in_=kH[g])
                        nc.scalar.dma_start(out=v_sb, in_=vH[g])
                Vs[g] = v_sb
                P = p_pool.tile([128, B, TK * QL], BF, tag="p", name=f"P{g}")
                Ps[g] = P
                if CFG["slot_gp"]:
                    T1s[g] = tree_pool.tile([128, 8, TK * QL], BF, tag="t1s",
                                            name=f"t1s_{g}")
                BS = TK * QL
                sched = SLOT_SCHED4 if CFG["zpe"] == 0 else SLOT_SCHED
                for t in range(TK):
                    for slot, bl in sched:
                        nb = len(bl)
                        if slot == "A":
                            s_ps = psA_pool.tile([128, 4 * QL], F32, tag="sa",
                                                 name=f"sa{g}_{t}")
                        elif CFG["zpe"] == 0:
                            s_ps = psB_pool.tile([128, 4 * QL], F32, tag="sb",
                                                 name=f"sb{g}_{t}")
                        else:
                            s_ps = psB_pool.tile([128, 2 * QL], F32, tag="sb",
                                                 name=f"sb{g}_{t}")
                        for i, b in enumerate(bl):
                            bo, bp = b % 2, b // 2
                            nc.tensor.matmul(
                                out=s_ps[:, i * QL:(i + 1) * QL],
                                lhsT=kt_sb[bo * 64:(bo + 1) * 64, bp,
                                           t * KT:(t + 1) * KT],
                                rhs=qt_sb[bo * 64:(bo + 1) * 64, bp, :],
                                start=(i % 2 == 0), stop=(i % 2 == 1),
                            )
                        p_ap = P[:, :, :]
                        off = p_ap.offset + bl[0] * BS + t * QL
                        if nb == 4:
                            free = [[(bl[2] - bl[0]) * BS, 2],
                                    [(bl[1] - bl[0]) * BS, 2], [1, QL]]
                            in_ap = s_ps[:, :].rearrange(
                                "p (o i q) -> p o i q", o=2, i=2)
                        else:
                            free = [[(bl[1] - bl[0]) * BS, 2], [1, QL]]
                            in_ap = s_ps[:, :].rearrange(
                                "p (i q) -> p i q", i=2)
                        out_ap = bass.AP(tensor=p_ap.tensor, offset=off,
                                         ap=[p_ap.ap[0]] + free)
                        nc.scalar.activation(
                            out=out_ap, in_=in_ap,
                            func=mybir.ActivationFunctionType.Exp,
                            scale=SCALE,
                        )
                        if CFG["slot_gp"]:
                            # pair-add this slot's 4 batches; GPSIMD for
                            # slots in gp_slots, DVE otherwise
                            si = sched.index((slot, bl))
                            tcs = slice(t * QL, (t + 1) * QL)
                            a = bl[0]
                            eng = (nc.gpsimd if si in CFG["gp_slots"]
                                   else nc.vector)
                            eng.tensor_tensor(
                                out=T1s[g][:, 2 * si:2 * si + 2, tcs],
                                in0=P[:, a:a + 2, tcs],
                                in1=P[:, a + 2:a + 4, tcs],
                                op=mybir.AluOpType.add)
                        if chunk is not None:
                            chunk()

            def emit_s2(g, lo=0, w=TK * QL, tag=""):
                """zsum + recip for columns [lo, lo+w) of group g -> Rb."""
                P = Ps[g]
                cs = slice(lo, lo + w)
                ZPEc = CFG["zpe"]
                m = B - ZPEc
                if CFG["slot_gp"]:
                    t1 = T1s[g][:, :, cs]
                else:
                    t1t = tree_pool.tile([128, m // 2, w], BF, tag=f"t1{tag}",
                                         name=f"t1{tag}_{g}")
                    l1_eng = nc.gpsimd if CFG["l1_gp"] else nc.vector
                    l1_eng.tensor_tensor(
                        out=t1t[:, :, :], in0=P[:, ZPEc::2, cs],
                        in1=P[:, ZPEc + 1::2, cs], op=mybir.AluOpType.add)
                    t1 = t1t[:, :, :]
                lvl, lw, li = t1, m // 2, 2
                stop_w = 2 if ZPEc == 0 else 1
                while lw > stop_w:
                    nxt = tree_pool.tile([128, lw // 2, w], BF,
                                         tag=f"t{li}{tag}",
                                         name=f"t{li}{tag}_{g}")
                    nc.vector.tensor_tensor(
                        out=nxt[:, :, :], in0=lvl[:, 0::2, :],
                        in1=lvl[:, 1::2, :], op=mybir.AluOpType.add)
                    lvl, lw, li = nxt[:, :, :], lw // 2, li + 1
                Zf = r_pool.tile([128, w], F32, tag=f"zf{tag}",
                                 name=f"zf{tag}_{g}")
                if ZPEc == 0:
                    nc.vector.tensor_tensor(out=Zf[:, :], in0=lvl[:, 0, :],
                                            in1=lvl[:, 1, :],
                                            op=mybir.AluOpType.add)
                else:
                    Zp = psZ_pool.tile([128, TK * QL], F32, tag="z",
                                       name=f"z{tag}{g}")
                    for j in range(ZPEc):
                        nc.tensor.matmul(
                            out=Zp[:, cs], lhsT=id_sb[:, :], rhs=P[:, j, cs],
                            start=(j == 0), stop=(j == ZPEc - 1),
                        )
                    nc.vector.tensor_tensor(out=Zf[:, :], in0=Zp[:, cs],
                                            in1=lvl[:, 0, :],
                                            op=mybir.AluOpType.add)
                Rb = r_pool.tile([128, w], BF, tag=f"rb{tag}",
                                 name=f"rb{tag}_{g}")
                if CFG["recip_bf"]:
                    from concourse.dve_ops import (
                        RECIP_APPROX_FAST_CONSTS, RECIPROCAL_APPROX_FAST)
                    c = RECIP_APPROX_FAST_CONSTS
                    nc.vector._custom_dve(
                        RECIPROCAL_APPROX_FAST, out=Rb[:, :], in0=Zf[:, :],
                        s0=c["s0"], s1=c["s1"], imm2=c["imm2"])
                else:
                    Rf = r_pool.tile([128, w], F32, tag=f"rf{tag}",
                                     name=f"rf{tag}_{g}")
                    nc.vector.reciprocal_approx_fast(out=Rf[:, :], in_=Zf[:, :])
                    nc.vector.tensor_copy(out=Rb[:, :], in_=Rf[:, :])
                return Rb

            def emit_s3(g, Rb, lo=0, w=TK * QL):
                P = Ps[g]
                cs = slice(lo, lo + w)
                A = As.get(g)
                if A is None:
                    A = a_pool.tile([128, B, TK * QL], BF, tag="a",
                                    name=f"A{g}")
                    As[g] = A
                rb_ap = Rb[:, :]
                rb_bcast = bass.AP(tensor=rb_ap.tensor, offset=rb_ap.offset,
                                   ap=[rb_ap.ap[0], [0, B], rb_ap.ap[1]])
                nc.vector.tensor_mul(out=A[:, :, cs], in0=P[:, :, cs],
                                     in1=rb_bcast)

            def s4_matmuls(g, first, last, ts=tuple(range(TK))):
                A, v_sb = As[g], Vs[g]
                for b in range(B):
                    bo, bp = b % 2, b // 2
                    for t in ts:
                        yield dict(
                            out=outacc[bp // 2][bo * 64:(bo + 1) * 64,
                                                (bp % 2) * QL:(bp % 2 + 1) * QL],
                            lhsT=v_sb[:, t, b, :],
                            rhs=A[:, b, t * QL:(t + 1) * QL],
                            start=(first and t == 0 and bp % 2 == 0),
                            stop=(last and t == TK - 1 and bp % 2 == 1),
                            # CoreSim's group tracker is partition-base
                            # blind; data semantics verified separately.
                            skip_group_check=True,
                        )

            def emit_s4(g, first, last, ts=tuple(range(TK))):
                for kw in s4_matmuls(g, first, last, ts):
                    nc.tensor.matmul(**kw)

            def release(g):
                Ps.pop(g, None), As.pop(g, None), Vs.pop(g, None)
                T1s.pop(g, None)

            import contextlib
            rep_ctx = tc.For_i(0, repeat, 1) if repeat > 1 else contextlib.nullcontext()
            with rep_ctx:
                if CFG["ilv"]:
                    for g in range(NG):
                        if g >= 2:
                            emit_s3(g - 2, Rz.pop(g - 2))
                            mm2_iter = s4_matmuls(g - 2, first=(g == 2),
                                                  last=False)
                            def chunk(it=mm2_iter):
                                for _ in range(4):
                                    kw = next(it, None)
                                    if kw is not None:
                                        nc.tensor.matmul(**kw)
                            emit_s1(g, chunk=chunk)
                            for kw in mm2_iter:
                                nc.tensor.matmul(**kw)
                            release(g - 2)
                        else:
                            emit_s1(g)
                        if g >= 1:
                            Rz[g - 1] = emit_s2(g - 1)
                    GL = NG - 1
                    emit_s3(GL - 1, Rz.pop(GL - 1))
                    emit_s4(GL - 1, first=False, last=False)
                    release(GL - 1)
                    Rb0 = emit_s2(GL, lo=0, w=QL, tag="h0")
                    emit_s3(GL, Rb0, lo=0, w=QL)
                    emit_s4(GL, first=False, last=False, ts=(0,))
                    Rb1 = emit_s2(GL, lo=QL, w=QL, tag="h1")
                    emit_s3(GL, Rb1, lo=QL, w=QL)
                    emit_s4(GL, first=False, last=True, ts=(1,))
                    release(GL)
                elif not CFG["tsplit"]:
                    for g in range(NG):
                        emit_s1(g)
                        if CFG["order"] == "s34_first":
                            if g >= 2:
                                emit_s3(g - 2, Rz.pop(g - 2))
                                emit_s4(g - 2, first=(g == 2), last=False)
                                release(g - 2)
                            if g >= 1:
                                Rz[g - 1] = emit_s2(g - 1)
                        else:
                            if g >= 1:
                                Rz[g - 1] = emit_s2(g - 1)
                            if g >= 2:
                                emit_s3(g - 2, Rz.pop(g - 2))
                                emit_s4(g - 2, first=(g == 2), last=False)
                                release(g - 2)
                    GL = NG - 1
                    emit_s3(GL - 1, Rz.pop(GL - 1))
                    emit_s4(GL - 1, first=False, last=False)
                    release(GL - 1)
                    Rb0 = emit_s2(GL, lo=0, w=QL, tag="h0")
                    emit_s3(GL, Rb0, lo=0, w=QL)
                    emit_s4(GL, first=False, last=False, ts=(0,))
                    Rb1 = emit_s2(GL, lo=QL, w=QL, tag="h1")
                    emit_s3(GL, Rb1, lo=QL, w=QL)
                    emit_s4(GL, first=False, last=True, ts=(1,))
                    release(GL)
                else:
                    # unit pipeline over (g, th): S2/S3/S4 at t-half grain
                    def s2u(g, th):
                        Rz[(g, th)] = emit_s2(g, lo=th * QL, w=QL,
                                              tag=f"u{th}")
                    def s34u(g, th, first, last):
                        emit_s3(g, Rz.pop((g, th)), lo=th * QL, w=QL)
                        emit_s4(g, first=first, last=last, ts=(th,))
                    for g in range(NG):
                        emit_s1(g)
                        if g >= 2:
                            s34u(g - 2, 0, first=(g == 2), last=False)
                        if g >= 1:
                            s2u(g - 1, 0)
                        if g >= 2:
                            s34u(g - 2, 1, first=(g == 2), last=False)
                        if g >= 1:
                            s2u(g - 1, 1)
                    GL = NG - 1
                    s34u(GL - 1, 0, first=False, last=False)
                    s2u(GL, 0)
                    s34u(GL - 1, 1, first=False, last=False)
                    s2u(GL, 1)
                    s34u(GL, 0, first=False, last=False)
                    s34u(GL, 1, first=False, last=True)
                    for g in range(NG):
                        release(g)

                out_sb = singles.tile([128, BP, QL],
                                      BF if CFG["obf"] else F32,
                                      name="out_sb")
                for i in range(BP // 2):
                    nc.vector.tensor_copy(
                        out=out_sb[:, 2 * i:2 * i + 2, :],
                        in_=outacc[i][:, :].rearrange("p (j q) -> p j q", j=2),
                    )
                nc.sync.dma_start(out=outH[:, :, :], in_=out_sb)
                if CFG["g0pin"]:
                    # refill g0's pinned K/V for the next repeat iteration;
                    # the DMA flies across the loop barrier. Dead weight for
                    # the single-shot run (overlaps the out DMA).
                    nc.sync.dma_start(out=kt_pin, in_=kH[0])
                    nc.scalar.dma_start(out=v_pin, in_=vH[0])

    nc.finalize()
    return nc


_NC_CACHE = None


def _get_program():
    global _NC_CACHE
    if _NC_CACHE is None:
        _NC_CACHE = build_program()
    return _NC_CACHE


def make_in_maps(queries, keys, values):
    """Host-side staging into SBUF partition-images (bf16)."""
    kHt = np.ascontiguousarray(
        keys.reshape(BP, 2, NG, GK, D).transpose(2, 1, 4, 0, 3)
    ).reshape(NG, 128, BP, GK).astype(bf16)
    vHt = np.ascontiguousarray(
        values.reshape(B, NG, TK, KT, D).transpose(1, 3, 2, 0, 4)
    ).astype(bf16)
    in_maps = []
    for c in range(NCORES):
        qs = queries[:, c * QL:(c + 1) * QL, :]
        qHc = np.ascontiguousarray(
            qs.reshape(BP, 2, QL, D).transpose(1, 3, 0, 2)
        ).reshape(128, BP, QL).astype(bf16)
        in_maps.append({"qH": qHc, "kH": kHt, "vH": vHt})
    return in_maps


def assemble_output(results):
    out = np.empty((B, N, D), dtype=np.float32)
    for c, res in enumerate(results):
        oc = res["outH"].astype(np.float32)
        oc = oc.reshape(2, D, BP, QL).transpose(2, 0, 3, 1)
        out[:, c * QL:(c + 1) * QL, :] = oc.reshape(B, QL, D)
    return out


def kernel(queries, keys, values):
    nc = _get_program()
    in_maps = make_in_maps(queries, keys, values)
    res = run_bass_kernel_spmd(nc, in_maps, core_ids=list(range(NCORES)))
    return assemble_output(res.results)
